# revision 12
# baseline (speedup 1.0000x reference)
"""Distributed causal multi-head attention for 8 TRN2 NeuronCores.

Problem: x[2, 2048, 1024], 16 heads x 64 dim, causal softmax attention,
output projection. Sharding: tensor-parallel over (batch, head-group):
core c handles batch c//4 and heads [4*(c%4), 4*(c%4)+4). Each core
computes its 4 heads' attention plus the partial output projection
(sum over its heads); the host sums the 4 partials per batch.

On-device layout strategy (no transposes anywhere on device):
  - host feeds xT = x[b].T               [D=1024, S=2048]
  - wq/wk/wv = W[heads] as [D, 256]      (d-major, head-major columns)
  - wo_h     = W_O slice per head        [64, 1024]
  - Q^T/K^T computed as [head-pair 128, S]; V as [p, 65*4] with a ones
    column folded per head so the attention-value matmul also produces
    the softmax denominator row.
  - scores tile = K^T.T @ Q^T -> [p=128, q=512] in PSUM; causal mask is
    an additive tril [128,128] applied only on true-diagonal blocks;
    fully-masked blocks are zeroed post-exp.
  - z^T accumulated in PSUM [65, 512] per head (row 64 = denominator l).
  - normalization: r = 1/l broadcast across partitions, z * r -> zn.
  - out[q,1024] = sum_h zn_h.T @ wo_h, accumulated in PSUM over heads.

Matmul compute dtype: float32r (full-rate on TRN2, ~1e-4 rel err);
accumulation fp32 in PSUM.
"""

import sys

if "/opt/trn_rl_repo" not in sys.path:
    sys.path.insert(0, "/opt/trn_rl_repo")

import numpy as np

import concourse.bass as bass
import concourse.mybir as mybir
import concourse.tile as tile
from concourse.bass_utils import run_bass_kernel_spmd

B = 2
S = 2048
D = 1024
NH = 16
DH = 64
N_CORES = 8
HPC = 4          # heads per core
HL = HPC * DH    # 256 local head dims
QC = 512         # q-chunk width
NQC = S // QC
NEG = -30000.0   # additive mask value; exp(NEG/8) == 0 in f32

F32 = mybir.dt.float32
F32R = mybir.dt.float32r
EXP = mybir.ActivationFunctionType.Exp


def _split_multiwait(nc, max_waits=1):
    """Walrus (CoreV3) rejects instructions carrying more than one sync
    wait; split extras into single-wait nops inserted before, same engine."""
    for f in nc.m.functions:
        for blk in f.blocks:
            insts = blk.instructions
            idx = 0
            while idx < len(insts):
                inst = insts[idx]
                si = getattr(inst, "sync_info", None)
                waits = list(si.on_wait) if si is not None else []
                if len(waits) > max_waits:
                    extra, keep = waits[:-max_waits], waits[-max_waits:]
                    si.on_wait = keep
                    for j, w in enumerate(extra):
                        nop = mybir.InstNoOp(
                            name=f"{inst.name}_sw{j}",
                            engine=inst.engine,
                            sync_info=mybir.SyncInfo(on_wait=[w], on_update=[]),
                            bass_nofuse=True,
                        )
                        insts.insert(idx, nop)
                        idx += 1
                idx += 1


def build_nc(stage=3):
    """stage 1: projections only (QT dumped to out); 2: + attention loop
    (zn dumped); 3: full kernel."""
    nc = bass.Bass("TRN2", target_bir_lowering=False, debug=False, num_devices=N_CORES)

    xT_d = nc.declare_dram_parameter("xT", [D, S], F32R, isOutput=False)
    wq_d = nc.declare_dram_parameter("wq", [D, HL], F32R, isOutput=False)
    wk_d = nc.declare_dram_parameter("wk", [D, HL], F32R, isOutput=False)
    wv_d = nc.declare_dram_parameter("wv", [D, HL], F32R, isOutput=False)
    wo_d = nc.declare_dram_parameter("wo", [HL, D], F32R, isOutput=False)
    mask_d = nc.declare_dram_parameter("mask", [128, 128], F32, isOutput=False)
    out_d = nc.declare_dram_parameter("out", [S, D], F32, isOutput=True)

    with tile.TileContext(nc) as tc:
        with (
            tc.tile_pool(name="live_sb", bufs=1) as live_sb,
            tc.tile_pool(name="att_sb", bufs=1) as att_sb,
        ):
            # Tensors that live through the whole kernel.
            QT = [live_sb.tile([128, S], F32R, tag=f"QT{hc}", name=f"QT{hc}") for hc in range(2)]
            KT = [live_sb.tile([128, S], F32R, tag=f"KT{hc}", name=f"KT{hc}") for hc in range(2)]
            # V with a ones column per head: 16 p-chunks x [V0|1|V1|1|V2|1|V3|1]
            V_sb = live_sb.tile([128, 16 * (HPC * 65)], F32R, tag="V", name="V")
            wo_t = [live_sb.tile([64, D], F32R, tag=f"wo{h}", name=f"wo{h}") for h in range(HPC)]
            mask_t = live_sb.tile([128, 128], F32, tag="mask", name="mask")

            # f32r tiles cannot be memset directly (walrus ISA check); build
            # ones in f32 and round via tensor_copy.
            ones_f = live_sb.tile([128, 64], F32, tag="ones_f", name="ones_f")
            nc.vector.memset(ones_f[:, :], 1.0)
            ones64 = live_sb.tile([1, 64], F32R, tag="ones64", name="ones64")
            nc.vector.tensor_copy(ones64[:, :], ones_f[0:1, :])

            nc.sync.dma_start(out=mask_t[:, :], in_=mask_d[:, :])
            for h in range(HPC):
                nc.sync.dma_start(
                    out=wo_t[h][:, :], in_=wo_d[h * DH:(h + 1) * DH, :]
                )

            # ---- Phase 1: projections (xT and w tiles scoped here) ----
            with (
                tc.tile_pool(name="xw_sb", bufs=1) as xw_sb,
                tc.tile_pool(name="proj_ps", bufs=4, space="PSUM") as proj_ps,
            ):
                xT_t = []
                for di in range(8):
                    t = xw_sb.tile([128, S], F32R, tag=f"x{di}", name=f"x{di}")
                    nc.sync.dma_start(out=t[:, :], in_=xT_d[di * 128:(di + 1) * 128, :])
                    xT_t.append(t)
                w_t = {}
                for name, dram in (("wq", wq_d), ("wk", wk_d), ("wv", wv_d)):
                    tiles = []
                    for di in range(8):
                        t = xw_sb.tile([128, HL], F32R, tag=f"{name}{di}", name=f"{name}{di}")
                        nc.sync.dma_start(
                            out=t[:, :], in_=dram[di * 128:(di + 1) * 128, :]
                        )
                        tiles.append(t)
                    w_t[name] = tiles

                # Q^T, K^T: [head-pair 128, S]
                for wname, dst in (("wq", QT), ("wk", KT)):
                    for hc in range(2):
                        for qt in range(4):
                            ps = proj_ps.tile([128, 512], F32, tag="pp", name="pp")
                            for di in range(8):
                                nc.tensor.matmul(
                                    ps[:, :],
                                    w_t[wname][di][:, hc * 128:(hc + 1) * 128],
                                    xT_t[di][:, qt * 512:(qt + 1) * 512],
                                    start=(di == 0),
                                    stop=(di == 7),
                                )
                            nc.vector.tensor_copy(
                                dst[hc][:, qt * 512:(qt + 1) * 512], ps[:, :]
                            )

                # V: [p, h] per p-chunk, interleaved with ones columns
                for pc in range(16):
                    ps = proj_ps.tile([128, 512], F32, tag="pp", name="pp")
                    for di in range(8):
                        nc.tensor.matmul(
                            ps[:, :HL],
                            xT_t[di][:, pc * 128:(pc + 1) * 128],
                            w_t["wv"][di][:, :],
                            start=(di == 0),
                            stop=(di == 7),
                        )
                    base = pc * (HPC * 65)
                    for h in range(HPC):
                        nc.vector.tensor_copy(
                            V_sb[:, base + h * 65: base + h * 65 + 64],
                            ps[:, h * 64:(h + 1) * 64],
                        )
                        nc.vector.tensor_copy(
                            V_sb[:, base + h * 65 + 64: base + h * 65 + 65],
                            ones_f[:, 0:1],
                        )

            if stage == 1:
                dbg = att_sb.tile([128, S], F32, tag="dbg", name="dbg")
                nc.vector.tensor_copy(dbg[:, :], QT[0][:, :])
                nc.sync.dma_start(out=out_d[0:128, :], in_=dbg[:, 0:1024])
                nc.sync.dma_start(out=out_d[128:256, :], in_=dbg[:, 1024:2048])

            # ---- Phase 2: attention + output projection ----
            with (
                tc.tile_pool(name="z_ps", bufs=4, space="PSUM") as z_ps,
                tc.tile_pool(name="sc_ps", bufs=2, space="PSUM") as sc_ps,
                tc.tile_pool(name="o_ps", bufs=2, space="PSUM") as o_ps,
            ):
                for qc in range(NQC if stage >= 2 else 0):
                    q0 = qc * QC
                    npt = q0 // 128 + 4
                    zt = [z_ps.tile([65, 512], F32, tag="z", name="z") for _ in range(HPC)]
                    for pt in range(npt):
                        p0 = pt * 128
                        jj = pt - q0 // 128  # >=0 means diagonal region
                        for h in range(HPC):
                            hc, ho = h // 2, (h % 2) * 64
                            sc = sc_ps.tile([128, 512], F32, tag="sc", name="sc")
                            nc.tensor.matmul(
                                sc[:, :],
                                KT[hc][ho:ho + 64, p0:p0 + 128],
                                QT[hc][ho:ho + 64, q0:q0 + QC],
                                start=True,
                                stop=True,
                            )
                            if jj >= 0:
                                nc.vector.tensor_add(
                                    sc[:, jj * 128:(jj + 1) * 128],
                                    sc[:, jj * 128:(jj + 1) * 128],
                                    mask_t[:, :],
                                )
                            if jj > 0:
                                # fully-masked blocks: push scores to -inf so
                                # exp yields exact zeros
                                nc.vector.tensor_scalar_add(
                                    sc[:, 0:jj * 128], sc[:, 0:jj * 128], NEG
                                )
                            P = att_sb.tile([128, 512], F32R, tag="P", name="P")
                            nc.scalar.activation(P[:, :], sc[:, :], EXP, scale=0.125)
                            nc.tensor.matmul(
                                zt[h][:, :],
                                V_sb[:, pt * (HPC * 65) + h * 65: pt * (HPC * 65) + (h + 1) * 65],
                                P[:, :],
                                start=(pt == 0),
                                stop=(pt == npt - 1),
                            )

                    # normalize: zn_h = z_h / l_h. r=1/l is broadcast across
                    # partitions via a K=1 outer-product matmul (ones64.T @ r).
                    zn = []
                    for h in range(HPC):
                        l_sb = att_sb.tile([1, 512], F32, tag="l", name="l")
                        nc.vector.tensor_copy(l_sb[:, :], zt[h][64:65, :])
                        r_f = att_sb.tile([1, 512], F32, tag="r_f", name="r_f")
                        nc.vector.reciprocal(r_f[:, :], l_sb[:, :])
                        r_sb = att_sb.tile([1, 512], F32R, tag="r", name="r")
                        nc.vector.tensor_copy(r_sb[:, :], r_f[:, :])
                        rb_ps = sc_ps.tile([64, 512], F32, tag="sc", name="rb_ps")
                        nc.tensor.matmul(
                            rb_ps[:, :], ones64[:, :], r_sb[:, :],
                            start=True, stop=True,
                        )
                        rb = att_sb.tile([64, 512], F32, tag="rb", name="rb")
                        nc.scalar.copy(rb[:, :], rb_ps[:, :])
                        znh = att_sb.tile([64, 512], F32R, tag=f"zn{h}", name=f"zn{h}")
                        nc.vector.tensor_mul(znh[:, :], zt[h][0:64, :], rb[:, :])
                        zn.append(znh)

                    if stage == 2:
                        for h in range(HPC):
                            nc.sync.dma_start(
                                out=out_d[q0:q0 + 64, h * 512:(h + 1) * 512]
                                if False else out_d[q0 + h * 64:q0 + (h + 1) * 64, 0:512],
                                in_=zn[h][:, :].bitcast(F32),
                            )
                        continue

                    # out[q0:q0+512, :] = sum_h zn_h.T @ wo_h
                    for qs in range(4):
                        for dm in range(2):
                            ps = o_ps.tile([128, 512], F32, tag="o", name="o")
                            for h in range(HPC):
                                nc.tensor.matmul(
                                    ps[:, :],
                                    zn[h][:, qs * 128:(qs + 1) * 128],
                                    wo_t[h][:, dm * 512:(dm + 1) * 512],
                                    start=(h == 0),
                                    stop=(h == HPC - 1),
                                )
                            ot = att_sb.tile([128, 512], F32, tag="ot", name="ot")
                            nc.vector.tensor_copy(ot[:, :], ps[:, :])
                            nc.sync.dma_start(
                                out=out_d[q0 + qs * 128: q0 + (qs + 1) * 128,
                                          dm * 512:(dm + 1) * 512],
                                in_=ot[:, :],
                            )

    _split_multiwait(nc)
    return nc


def _prep_in_maps(x, W_K, W_Q, W_V, W_O):
    x = np.asarray(x, dtype=np.float32)
    W_K = np.asarray(W_K, dtype=np.float32)
    W_Q = np.asarray(W_Q, dtype=np.float32)
    W_V = np.asarray(W_V, dtype=np.float32)
    W_O = np.asarray(W_O, dtype=np.float32)

    pp, qq = np.meshgrid(np.arange(128), np.arange(128), indexing="ij")
    mask = np.where(qq >= pp, 0.0, NEG).astype(np.float32)

    in_maps = []
    for c in range(N_CORES):
        b, g = c // 4, c % 4
        hs = slice(HPC * g, HPC * g + HPC)
        xT = np.ascontiguousarray(x[b].T)
        wq = np.ascontiguousarray(W_Q[hs].transpose(2, 0, 1).reshape(D, HL))
        wk = np.ascontiguousarray(W_K[hs].transpose(2, 0, 1).reshape(D, HL))
        wv = np.ascontiguousarray(W_V[hs].transpose(2, 0, 1).reshape(D, HL))
        wo = np.ascontiguousarray(W_O[:, HL * g:HL * g + HL].T)
        in_maps.append(
            {"xT": xT, "wq": wq, "wk": wk, "wv": wv, "wo": wo, "mask": mask}
        )
    return in_maps


_NC_CACHE = None


def _get_nc():
    global _NC_CACHE
    if _NC_CACHE is None:
        _NC_CACHE = build_nc()
    return _NC_CACHE


def _run(x, W_K, W_Q, W_V, W_O, trace=False):
    nc = _get_nc()
    in_maps = _prep_in_maps(x, W_K, W_Q, W_V, W_O)
    res = run_bass_kernel_spmd(
        nc, in_maps, core_ids=list(range(N_CORES)), trace=trace
    )
    partials = np.stack([res.results[c]["out"] for c in range(N_CORES)])
    out = np.empty((B, S, D), dtype=np.float32)
    out[0] = partials[0:4].sum(axis=0)
    out[1] = partials[4:8].sum(axis=0)
    return out, res


def kernel(x, W_K, W_Q, W_V, W_O):
    out, _ = _run(x, W_K, W_Q, W_V, W_O, trace=False)
    return out


def run_traced(x, W_K, W_Q, W_V, W_O):
    """For test.py: returns (out, BassKernelResults with exec_time_ns)."""
    import types

    if "antenv.axon_hooks" not in sys.modules:
        try:
            from trn_agent_boot.trn_boot import _ntff_profile_via_ctypes

            hook = _ntff_profile_via_ctypes("/opt/axon/libaxon_pjrt.so")
            mod = types.ModuleType("antenv.axon_hooks")
            mod.get_axon_ntff_profile_hook = lambda: hook
            mod.set_axon_ntff_profile_hook = lambda h: None
            sys.modules["antenv.axon_hooks"] = mod
        except Exception:
            pass
    return _run(x, W_K, W_Q, W_V, W_O, trace=True)
